# revision 9
# baseline (speedup 1.0000x reference)
"""Trainium2 Bass kernel for the Chunked TTT layer (SwiGLU fast-weight MLP
with per-chunk scalar weight decay).

Reference semantics (B=4, S=4096, H=1024, F=4096, CHUNK=512, LR=1e-3):
    for each chunk k (sequential):
        out_k  = silu(x_k @ a0) * (x_k @ a2) @ a1
        loss_k = mean((out_k - x_k)^2)
        a*     = a* * (1 - LR*loss_k)
    returns (out [B,S,H], mean_k loss_k)

Key algebraic transform: the decay is a pure scalar, so with D_k = prod of
decays, out_k = D_k^2 * (silu(D_k*u_k) * v_k) @ w1 where u_k = x_k@w0 and
v_k = x_k@w2 use the ORIGINAL weights. Weights therefore stay constant on
device and only a [128,1]-broadcast scalar D propagates between chunks.

Sharding: 8-way split of the 2048 rows (B x CHUNK) of every chunk; weights
replicated. The only cross-core dependency is the per-chunk scalar SSE,
handled with a tiny AllReduce that overlaps the next chunk's D-independent
matmuls.
"""

import numpy as np
import ml_dtypes

# ---- problem constants (hardcoded per contest rules) ----
B, S, H, F = 4, 4096, 1024, 4096
CHUNK = 512
NC_CHUNKS = S // CHUNK          # 8 sequential chunks
LR = 0.001
CORES = 8
R = (B * CHUNK) // CORES        # 256 rows per core per chunk
NH = H // 128                   # 8 h-tiles
NF = F // 128                   # 32 f-tiles
RO = R // 128                   # 2 row blocks
HSL = H // 512                  # 2 output column slices
LOSS_DENOM = float(B * CHUNK * H)   # 2097152 elements per chunk loss
W0_RESIDENT = True
import os as _os
ABLATE_AR = _os.environ.get("ABLATE_AR", "0") == "1"      # skip collectives (wrong numerics)
ABLATE_PAR = _os.environ.get("ABLATE_PAR", "0") == "1"    # skip partition_all_reduce

_BF16 = ml_dtypes.bfloat16

_nc_cache = {}


def _build_nc():
    import concourse.bass as bass
    import concourse.bass_isa as bass_isa
    import concourse.mybir as mybir
    import concourse.tile as tile
    from concourse import bacc

    f32 = mybir.dt.float32
    bf16 = mybir.dt.bfloat16
    Alu = mybir.AluOpType
    Act = mybir.ActivationFunctionType

    nc = bacc.Bacc(None, target_bir_lowering=False)

    # ---- I/O ----
    # w0t / w2s: per f-strip, per h-tile 128x128 blocks; partition = h-inner.
    w0t = nc.declare_dram_parameter("w0t", [128, NF, NH, 128], bf16, isOutput=False)
    w2s = nc.declare_dram_parameter("w2s", [NF, 128, NH, 128], bf16, isOutput=False)
    # w1s: per f-tile [128, H] natural slabs.
    w1s = nc.declare_dram_parameter("w1s", [NF, 128, H], bf16, isOutput=False)
    # xT per chunk: [hi, h, r]
    xt = nc.declare_dram_parameter("xt", [NC_CHUNKS, 128, NH, R], bf16, isOutput=False)
    # x natural rows per chunk: [ri, ro, hcol] (for the loss)
    xn = nc.declare_dram_parameter("xn", [NC_CHUNKS, 128, RO, H], bf16, isOutput=False)

    out = nc.declare_dram_parameter("out", [NC_CHUNKS, 128, RO, H], f32, isOutput=True)
    sse_out = nc.declare_dram_parameter("sse", [NC_CHUNKS], f32, isOutput=True)

    core_ids = list(range(CORES))
    neg_c = -(LR / LOSS_DENOM)

    with tile.TileContext(nc, num_cores=CORES) as tc:
        with (
            tc.tile_pool(name="persist", bufs=1) as persist,
            tc.tile_pool(name="w2pool", bufs=5) as w2pool,
            tc.tile_pool(name="w1pool", bufs=5) as w1pool,
            tc.tile_pool(name="xtpool", bufs=3) as xtpool,
            tc.tile_pool(name="xnpool", bufs=3) as xnpool,
            tc.tile_pool(name="uvpool", bufs=66) as uvpool,
            tc.tile_pool(name="hhpool", bufs=36) as hhpool,
            tc.tile_pool(name="spool", bufs=6) as spool,
            tc.tile_pool(name="outpool", bufs=6) as outpool,
            tc.tile_pool(name="diffpool", bufs=4) as diffpool,
            tc.tile_pool(name="small", bufs=4) as small,
            tc.tile_pool(name="d2pool", bufs=2) as d2pool,
            tc.tile_pool(name="upsum", bufs=2, space="PSUM") as upsum,
            tc.tile_pool(name="vpsum", bufs=2, space="PSUM") as vpsum,
            tc.tile_pool(name="opsum", bufs=4, space="PSUM") as opsum,
            tc.tile_pool(name="drampool", bufs=2, space="DRAM") as drampool,
        ):
            # ---- persistent state ----
            d_col = persist.tile([128, 1], f32)       # cumulative decay D_k
            nc.vector.memset(d_col[:], 1.0)
            d2_cur = persist.tile([128, 1], f32)      # D_k^2 for current chunk
            nc.vector.memset(d2_cur[:], 1.0)

            if W0_RESIDENT:
                w0_sb = persist.tile([128, NF, NH, 128], bf16)
                # split the 8MB load into 8 DMAs so it spreads across engines
                for g in range(8):
                    nc.sync.dma_start(
                        out=w0_sb[:, g * 4:(g + 1) * 4], in_=w0t[:, g * 4:(g + 1) * 4]
                    )

            # ---- helpers ----
            ustore = {}
            vstore = {}
            hstore = {}
            xt_tiles = {}
            xn_tiles = {}

            def prefetch_inputs(k):
                t = xtpool.tile([128, NH, R], bf16, tag="xt")
                nc.sync.dma_start(out=t[:], in_=xt[k])
                xt_tiles[k] = t
                t2 = xnpool.tile([128, RO, H], bf16, tag="xn")
                nc.sync.dma_start(out=t2[:], in_=xn[k])
                xn_tiles[k] = t2

            def mm12(k):
                """u,v = x_k @ w0, x_k @ w2 (transposed: [F, rows]), store bf16."""
                xt_t = xt_tiles[k]
                for f in range(NF):
                    if not W0_RESIDENT:
                        w0_t = w2pool.tile([128, NH, 128], bf16, tag="w0s")
                        nc.sync.dma_start(out=w0_t[:], in_=w0t[:, f])
                    w2_t = w2pool.tile([128, NH, 128], bf16, tag="w2s")
                    nc.sync.dma_start(out=w2_t[:], in_=w2s[f])

                    u_ps = upsum.tile([128, R], f32, tag="u_ps")
                    for h in range(NH):
                        lhsT = w0_sb[:, f, h, :] if W0_RESIDENT else w0_t[:, h, :]
                        nc.tensor.matmul(
                            u_ps[:], lhsT, xt_t[:, h, :],
                            start=(h == 0), stop=(h == NH - 1),
                        )
                    u_sb = uvpool.tile([128, R], bf16, tag="uv")
                    nc.scalar.copy(u_sb[:], u_ps[:])
                    ustore[(k, f)] = u_sb

                    v_ps = vpsum.tile([128, R], f32, tag="v_ps")
                    for h in range(NH):
                        nc.tensor.matmul(
                            v_ps[:], w2_t[:, h, :], xt_t[:, h, :],
                            start=(h == 0), stop=(h == NH - 1),
                        )
                    v_sb = uvpool.tile([128, R], bf16, tag="uv")
                    nc.vector.tensor_copy(v_sb[:], v_ps[:])
                    vstore[(k, f)] = v_sb

            # ---- pipeline warmup ----
            prefetch_inputs(0)
            prefetch_inputs(1)
            mm12(0)
            mm12(1)

            for k in range(NC_CHUNKS):
                # hh = silu(D*u) * v   (the outer D*D is folded into out-scale)
                for f in range(NF):
                    s_t = spool.tile([128, R], bf16, tag="s")
                    nc.scalar.activation(
                        s_t[:], ustore[(k, f)][:], Act.Silu, scale=d_col[:, 0:1]
                    )
                    hh = hhpool.tile([128, R], bf16, tag="hh")
                    nc.vector.tensor_tensor(
                        hh[:], s_t[:], vstore[(k, f)][:], Alu.mult
                    )
                    hstore[(k, f)] = hh
                    del ustore[(k, f)], vstore[(k, f)]

                # out = D^2 * (hh @ w1)   [rows, H]
                o_ps = []
                for m in range(RO):
                    row = []
                    for j in range(HSL):
                        t_ops = opsum.tile(
                            [128, 512], f32, tag="o_ps", name=f"ops_{k}_{m}_{j}"
                        )
                        row.append(t_ops)
                    o_ps.append(row)
                for f in range(NF):
                    w1_t = w1pool.tile([128, H], bf16, tag="w1")
                    nc.sync.dma_start(out=w1_t[:], in_=w1s[f])
                    hh = hstore[(k, f)]
                    for m in range(RO):
                        for j in range(HSL):
                            nc.tensor.matmul(
                                o_ps[m][j][:],
                                hh[:, m * 128:(m + 1) * 128],
                                w1_t[:, j * 512:(j + 1) * 512],
                                start=(f == 0), stop=(f == NF - 1),
                            )
                for f in range(NF):
                    del hstore[(k, f)]

                # scale by D^2, write out, and accumulate the squared loss
                xn_t = xn_tiles[k]
                sse4 = small.tile([128, RO * HSL], f32, tag="sse4")
                for m in range(RO):
                    for j in range(HSL):
                        o_sb = outpool.tile([128, 512], f32, tag="o_sb")
                        nc.vector.tensor_scalar(
                            o_sb[:], o_ps[m][j][:], d2_cur[:, 0:1], None, Alu.mult
                        )
                        nc.sync.dma_start(
                            out=out[k, :, m, j * 512:(j + 1) * 512], in_=o_sb[:]
                        )
                        dif = diffpool.tile([128, 512], f32, tag="dif")
                        nc.vector.tensor_tensor(
                            dif[:], o_sb[:], xn_t[:, m, j * 512:(j + 1) * 512],
                            Alu.subtract,
                        )
                        sq = diffpool.tile([128, 512], bf16, tag="sq")
                        nc.scalar.activation(
                            sq[:], dif[:], Act.Square,
                            accum_out=sse4[:, m * HSL + j: m * HSL + j + 1],
                        )

                sse_col = small.tile([128, 1], f32, tag="sse_col")
                nc.vector.tensor_reduce(
                    sse_col[:], sse4[:], mybir.AxisListType.X, Alu.add
                )
                # local cross-partition total, broadcast to all partitions
                sse_bc = small.tile([128, 1], f32, tag="sse_bc")
                if ABLATE_PAR:
                    nc.vector.tensor_copy(sse_bc[:], sse_col[:])
                else:
                    nc.gpsimd.partition_all_reduce(
                        sse_bc[:], sse_col[:], 128, bass_isa.ReduceOp.add
                    )

                # cross-core AllReduce of the (replicated) scalar
                ar_in = drampool.tile([128], f32, tag="ar_in")
                ar_out = drampool.tile([128], f32, tag="ar_out")
                nc.gpsimd.dma_start(out=ar_in[:], in_=sse_bc[:, 0])
                if ABLATE_AR:
                    nc.gpsimd.dma_start(out=ar_out[:], in_=ar_in[:])
                else:
                    nc.gpsimd.collective_compute(
                        "AllReduce",
                        Alu.add,
                        replica_groups=[core_ids],
                        ins=[ar_in.opt()],
                        outs=[ar_out.opt()],
                    )
                sse_ar = small.tile([128, 1], f32, tag="sse_ar")
                nc.gpsimd.dma_start(out=sse_ar[:, 0], in_=ar_out[:])
                nc.gpsimd.dma_start(out=sse_out[k:k + 1], in_=ar_out[0:1])

                # D_{k+1} = D_k * (1 - LR/DENOM * sse)
                decay = small.tile([128, 1], f32, tag="decay")
                nc.vector.tensor_scalar(
                    decay[:], sse_ar[:], neg_c, 1.0, Alu.mult, Alu.add
                )
                nc.vector.tensor_tensor(d_col[:], d_col[:], decay[:], Alu.mult)
                d2_nxt = d2pool.tile([128, 1], f32, tag="d2")
                nc.vector.tensor_tensor(d2_nxt[:], d_col[:], d_col[:], Alu.mult)
                d2_cur = d2_nxt

                # prefetch + D-independent matmuls for chunk k+2 (these keep
                # the PE busy while the AllReduce for chunk k is in flight)
                if k + 2 < NC_CHUNKS:
                    prefetch_inputs(k + 2)
                    mm12(k + 2)

    if not nc.is_finalized():
        nc.finalize()
    return nc


def _prep_inputs(x, w0, w1, w2):
    """Build per-core input maps (host-side sharding + layout)."""
    maps = []
    w0t = np.ascontiguousarray(
        w0.reshape(NH, 128, NF, 128).transpose(1, 2, 0, 3)
    ).astype(_BF16)  # [hi, f, h, fi]
    w2s = np.ascontiguousarray(
        w2.reshape(NH, 128, NF, 128).transpose(2, 1, 0, 3)
    ).astype(_BF16)  # [f, hi, h, fi]
    w1s = w1.reshape(NF, 128, H).astype(_BF16)

    xr = x.reshape(B, NC_CHUNKS, CHUNK, H)
    for core in range(CORES):
        b = core // 2
        c0 = (core % 2) * R
        xc = xr[b, :, c0:c0 + R, :]  # [nC, R, H] fp32
        # xt: [k, hi, h, r]
        xt = np.ascontiguousarray(
            xc.reshape(NC_CHUNKS, R, NH, 128).transpose(0, 3, 2, 1)
        ).astype(_BF16)
        # xn: [k, ri, ro, hcol]
        xn = np.ascontiguousarray(
            xc.reshape(NC_CHUNKS, RO, 128, H).transpose(0, 2, 1, 3)
        ).astype(_BF16)
        maps.append({"w0t": w0t, "w2s": w2s, "w1s": w1s, "xt": xt, "xn": xn})
    return maps


def _gather(results):
    out = np.empty((B, S, H), np.float32)
    xr = out.reshape(B, NC_CHUNKS, CHUNK, H)
    for core in range(CORES):
        b = core // 2
        c0 = (core % 2) * R
        oc = results[core]["out"]  # [nC, 128, RO, H]
        xr[b, :, c0:c0 + R, :] = oc.transpose(0, 2, 1, 3).reshape(NC_CHUNKS, R, H)
    sse = results[0]["sse"].astype(np.float64)
    losses = sse / LOSS_DENOM
    mean_loss = np.float32(losses.mean())
    return out, mean_loss


def kernel(x, w0, w1, w2, _trace=False):
    from concourse.bass_utils import run_bass_kernel_spmd

    if "nc" not in _nc_cache:
        _nc_cache["nc"] = _build_nc()
    nc = _nc_cache["nc"]

    in_maps = _prep_inputs(
        np.asarray(x, np.float32), np.asarray(w0, np.float32),
        np.asarray(w1, np.float32), np.asarray(w2, np.float32),
    )
    res = run_bass_kernel_spmd(nc, in_maps, list(range(CORES)), trace=_trace)
    out, mean_loss = _gather(res.results)
    if _trace:
        kernel.last_exec_time_ns = res.exec_time_ns
        kernel.last_results = res
    return out, mean_loss


# revision 11
# speedup vs baseline: 1.2309x; 1.2309x over previous
"""Trainium2 Bass kernel for the Chunked TTT layer (SwiGLU fast-weight MLP
with per-chunk scalar weight decay).

Reference semantics (B=4, S=4096, H=1024, F=4096, CHUNK=512, LR=1e-3):
    for each chunk k (sequential):
        out_k  = silu(x_k @ a0) * (x_k @ a2) @ a1
        loss_k = mean((out_k - x_k)^2)
        a*     = a* * (1 - LR*loss_k)
    returns (out [B,S,H], mean_k loss_k)

Key algebraic transform: the decay is a pure scalar, so with D_k = prod of
decays, out_k = D_k^2 * (silu(D_k*u_k) * v_k) @ w1 where u_k = x_k@w0 and
v_k = x_k@w2 use the ORIGINAL weights. Weights therefore stay constant on
device and only a [128,1]-broadcast scalar D propagates between chunks.

Sharding: 8-way split of the 2048 rows (B x CHUNK) of every chunk; weights
replicated. The only cross-core dependency is the per-chunk scalar SSE,
handled with a tiny AllReduce that overlaps the next chunk's D-independent
matmuls.
"""

import numpy as np
import ml_dtypes

# ---- problem constants (hardcoded per contest rules) ----
B, S, H, F = 4, 4096, 1024, 4096
CHUNK = 512
NC_CHUNKS = S // CHUNK          # 8 sequential chunks
LR = 0.001
CORES = 8
R = (B * CHUNK) // CORES        # 256 rows per core per chunk
NH = H // 128                   # 8 h-tiles
NF = F // 128                   # 32 f-tiles
RO = R // 128                   # 2 row blocks
HSL = H // 512                  # 2 output column slices
LOSS_DENOM = float(B * CHUNK * H)   # 2097152 elements per chunk loss
W0_RESIDENT = True
import os as _os
ABLATE_AR = _os.environ.get("ABLATE_AR", "0") == "1"      # skip collectives (wrong numerics)
ABLATE_PAR = _os.environ.get("ABLATE_PAR", "0") == "1"    # skip partition_all_reduce
ABLATE_LOSS = _os.environ.get("ABLATE_LOSS", "0") == "1"  # skip loss+AR+D chain entirely
ABLATE_TAIL = _os.environ.get("ABLATE_TAIL", "0") == "1"  # skip h'/MM3/out: mm12 only

_BF16 = ml_dtypes.bfloat16

_nc_cache = {}


def _build_nc():
    import concourse.bass as bass
    import concourse.bass_isa as bass_isa
    import concourse.mybir as mybir
    import concourse.tile as tile
    from concourse import bacc

    f32 = mybir.dt.float32
    bf16 = mybir.dt.bfloat16
    Alu = mybir.AluOpType
    Act = mybir.ActivationFunctionType

    nc = bacc.Bacc(None, target_bir_lowering=False)

    # ---- I/O ----
    # w0t / w2s: per f-strip, per h-tile 128x128 blocks; partition = h-inner.
    w0t = nc.declare_dram_parameter("w0t", [128, NF, NH, 128], bf16, isOutput=False)
    w2s = nc.declare_dram_parameter("w2s", [NF, 128, NH, 128], bf16, isOutput=False)
    # w1s: per f-tile [128, H] natural slabs.
    w1s = nc.declare_dram_parameter("w1s", [NF, 128, H], bf16, isOutput=False)
    # xT per chunk: [hi, h, r]
    xt = nc.declare_dram_parameter("xt", [NC_CHUNKS, 128, NH, R], bf16, isOutput=False)
    # x natural rows per chunk: [ri, ro, hcol] (for the loss)
    xn = nc.declare_dram_parameter("xn", [NC_CHUNKS, 128, RO, H], bf16, isOutput=False)

    out = nc.declare_dram_parameter("out", [NC_CHUNKS, 128, RO, H], f32, isOutput=True)
    sse_out = nc.declare_dram_parameter("sse", [NC_CHUNKS], f32, isOutput=True)

    core_ids = list(range(CORES))
    neg_c = -(LR / LOSS_DENOM)

    with tile.TileContext(nc, num_cores=CORES) as tc:
        with (
            tc.tile_pool(name="persist", bufs=1) as persist,
            tc.tile_pool(name="w2pool", bufs=5) as w2pool,
            tc.tile_pool(name="w1pool", bufs=5) as w1pool,
            tc.tile_pool(name="xtpool", bufs=3) as xtpool,
            tc.tile_pool(name="xnpool", bufs=3) as xnpool,
            tc.tile_pool(name="uvpool", bufs=66) as uvpool,
            tc.tile_pool(name="hhpool", bufs=36) as hhpool,
            tc.tile_pool(name="spool", bufs=6) as spool,
            tc.tile_pool(name="outpool", bufs=6) as outpool,
            tc.tile_pool(name="diffpool", bufs=4) as diffpool,
            tc.tile_pool(name="small", bufs=4) as small,
            tc.tile_pool(name="d2pool", bufs=2) as d2pool,
            tc.tile_pool(name="upsum", bufs=2, space="PSUM") as upsum,
            tc.tile_pool(name="vpsum", bufs=2, space="PSUM") as vpsum,
            tc.tile_pool(name="opsum", bufs=4, space="PSUM") as opsum,
            tc.tile_pool(name="drampool", bufs=2, space="DRAM") as drampool,
        ):
            # ---- persistent state ----
            d_col = persist.tile([128, 1], f32)       # cumulative decay D_k
            nc.vector.memset(d_col[:], 1.0)
            d2_cur = persist.tile([128, 1], f32)      # D_k^2 for current chunk
            nc.vector.memset(d2_cur[:], 1.0)

            if W0_RESIDENT:
                w0_sb = persist.tile([128, NF, NH, 128], bf16)
                # split the 8MB load into 8 DMAs so it spreads across engines
                for g in range(8):
                    nc.sync.dma_start(
                        out=w0_sb[:, g * 4:(g + 1) * 4], in_=w0t[:, g * 4:(g + 1) * 4]
                    )

            # ---- helpers ----
            ustore = {}
            vstore = {}
            hstore = {}
            xt_tiles = {}
            xn_tiles = {}

            def prefetch_inputs(k):
                t = xtpool.tile([128, NH, R], bf16, tag="xt")
                nc.sync.dma_start(out=t[:], in_=xt[k])
                xt_tiles[k] = t
                t2 = xnpool.tile([128, RO, H], bf16, tag="xn")
                nc.sync.dma_start(out=t2[:], in_=xn[k])
                xn_tiles[k] = t2

            def mm12(k):
                """u,v = x_k @ w0, x_k @ w2 (transposed: [F, rows]), store bf16."""
                xt_t = xt_tiles[k]
                for f in range(NF):
                    if not W0_RESIDENT:
                        w0_t = w2pool.tile([128, NH, 128], bf16, tag="w0s")
                        nc.sync.dma_start(out=w0_t[:], in_=w0t[:, f])
                    w2_t = w2pool.tile([128, NH, 128], bf16, tag="w2s")
                    nc.sync.dma_start(out=w2_t[:], in_=w2s[f])

                    u_ps = upsum.tile([128, R], f32, tag="u_ps")
                    for h in range(NH):
                        lhsT = w0_sb[:, f, h, :] if W0_RESIDENT else w0_t[:, h, :]
                        nc.tensor.matmul(
                            u_ps[:], lhsT, xt_t[:, h, :],
                            start=(h == 0), stop=(h == NH - 1),
                        )
                    u_sb = uvpool.tile([128, R], bf16, tag="uv")
                    nc.scalar.copy(u_sb[:], u_ps[:])
                    ustore[(k, f)] = u_sb

                    v_ps = vpsum.tile([128, R], f32, tag="v_ps")
                    for h in range(NH):
                        nc.tensor.matmul(
                            v_ps[:], w2_t[:, h, :], xt_t[:, h, :],
                            start=(h == 0), stop=(h == NH - 1),
                        )
                    v_sb = uvpool.tile([128, R], bf16, tag="uv")
                    nc.vector.tensor_copy(v_sb[:], v_ps[:])
                    vstore[(k, f)] = v_sb

            # ---- pipeline warmup ----
            prefetch_inputs(0)
            prefetch_inputs(1)
            mm12(0)
            mm12(1)

            for k in range(NC_CHUNKS):
                if ABLATE_TAIL:
                    for f in range(NF):
                        del ustore[(k, f)], vstore[(k, f)]
                    if k + 2 < NC_CHUNKS:
                        prefetch_inputs(k + 2)
                        mm12(k + 2)
                    continue
                # hh = silu(D*u) * v   (the outer D*D is folded into out-scale)
                for f in range(NF):
                    s_t = spool.tile([128, R], bf16, tag="s")
                    nc.scalar.activation(
                        s_t[:], ustore[(k, f)][:], Act.Silu, scale=d_col[:, 0:1]
                    )
                    hh = hhpool.tile([128, R], bf16, tag="hh")
                    nc.vector.tensor_tensor(
                        hh[:], s_t[:], vstore[(k, f)][:], Alu.mult
                    )
                    hstore[(k, f)] = hh
                    del ustore[(k, f)], vstore[(k, f)]

                # out = D^2 * (hh @ w1)   [rows, H]
                o_ps = []
                for m in range(RO):
                    row = []
                    for j in range(HSL):
                        t_ops = opsum.tile(
                            [128, 512], f32, tag="o_ps", name=f"ops_{k}_{m}_{j}"
                        )
                        row.append(t_ops)
                    o_ps.append(row)
                for f in range(NF):
                    w1_t = w1pool.tile([128, H], bf16, tag="w1")
                    nc.sync.dma_start(out=w1_t[:], in_=w1s[f])
                    hh = hstore[(k, f)]
                    for m in range(RO):
                        for j in range(HSL):
                            nc.tensor.matmul(
                                o_ps[m][j][:],
                                hh[:, m * 128:(m + 1) * 128],
                                w1_t[:, j * 512:(j + 1) * 512],
                                start=(f == 0), stop=(f == NF - 1),
                            )
                for f in range(NF):
                    del hstore[(k, f)]

                # scale by D^2, write out, and accumulate the squared loss
                xn_t = xn_tiles[k]
                sse4 = small.tile([128, RO * HSL], f32, tag="sse4")
                for m in range(RO):
                    for j in range(HSL):
                        o_sb = outpool.tile([128, 512], f32, tag="o_sb")
                        nc.vector.tensor_scalar(
                            o_sb[:], o_ps[m][j][:], d2_cur[:, 0:1], None, Alu.mult
                        )
                        nc.sync.dma_start(
                            out=out[k, :, m, j * 512:(j + 1) * 512], in_=o_sb[:]
                        )
                        dif = diffpool.tile([128, 512], f32, tag="dif")
                        nc.vector.tensor_tensor(
                            dif[:], o_sb[:], xn_t[:, m, j * 512:(j + 1) * 512],
                            Alu.subtract,
                        )
                        sq = diffpool.tile([128, 512], bf16, tag="sq")
                        nc.scalar.activation(
                            sq[:], dif[:], Act.Square,
                            accum_out=sse4[:, m * HSL + j: m * HSL + j + 1],
                        )

                if ABLATE_LOSS:
                    if k + 2 < NC_CHUNKS:
                        prefetch_inputs(k + 2)
                        mm12(k + 2)
                    continue
                sse_col = small.tile([128, 1], f32, tag="sse_col")
                nc.vector.tensor_reduce(
                    sse_col[:], sse4[:], mybir.AxisListType.X, Alu.add
                )
                # local cross-partition total, broadcast to all partitions
                sse_bc = small.tile([128, 1], f32, tag="sse_bc")
                if ABLATE_PAR:
                    nc.vector.tensor_copy(sse_bc[:], sse_col[:])
                else:
                    nc.gpsimd.partition_all_reduce(
                        sse_bc[:], sse_col[:], 128, bass_isa.ReduceOp.add
                    )

                # cross-core AllReduce of the (replicated) scalar
                ar_in = drampool.tile([128], f32, tag="ar_in")
                ar_out = drampool.tile([128], f32, tag="ar_out")
                nc.gpsimd.dma_start(out=ar_in[:], in_=sse_bc[:, 0])
                if ABLATE_AR:
                    nc.gpsimd.dma_start(out=ar_out[:], in_=ar_in[:])
                else:
                    nc.gpsimd.collective_compute(
                        "AllReduce",
                        Alu.add,
                        replica_groups=[core_ids],
                        ins=[ar_in.opt()],
                        outs=[ar_out.opt()],
                    )
                sse_ar = small.tile([128, 1], f32, tag="sse_ar")
                nc.gpsimd.dma_start(out=sse_ar[:, 0], in_=ar_out[:])
                nc.gpsimd.dma_start(out=sse_out[k:k + 1], in_=ar_out[0:1])

                # D_{k+1} = D_k * (1 - LR/DENOM * sse)
                decay = small.tile([128, 1], f32, tag="decay")
                nc.vector.tensor_scalar(
                    decay[:], sse_ar[:], neg_c, 1.0, Alu.mult, Alu.add
                )
                nc.vector.tensor_tensor(d_col[:], d_col[:], decay[:], Alu.mult)
                d2_nxt = d2pool.tile([128, 1], f32, tag="d2")
                nc.vector.tensor_tensor(d2_nxt[:], d_col[:], d_col[:], Alu.mult)
                d2_cur = d2_nxt

                # prefetch + D-independent matmuls for chunk k+2 (these keep
                # the PE busy while the AllReduce for chunk k is in flight)
                if k + 2 < NC_CHUNKS:
                    prefetch_inputs(k + 2)
                    mm12(k + 2)

    if not nc.is_finalized():
        nc.finalize()
    return nc


def _prep_inputs(x, w0, w1, w2):
    """Build per-core input maps (host-side sharding + layout)."""
    maps = []
    w0t = np.ascontiguousarray(
        w0.reshape(NH, 128, NF, 128).transpose(1, 2, 0, 3)
    ).astype(_BF16)  # [hi, f, h, fi]
    w2s = np.ascontiguousarray(
        w2.reshape(NH, 128, NF, 128).transpose(2, 1, 0, 3)
    ).astype(_BF16)  # [f, hi, h, fi]
    w1s = w1.reshape(NF, 128, H).astype(_BF16)

    xr = x.reshape(B, NC_CHUNKS, CHUNK, H)
    for core in range(CORES):
        b = core // 2
        c0 = (core % 2) * R
        xc = xr[b, :, c0:c0 + R, :]  # [nC, R, H] fp32
        # xt: [k, hi, h, r]
        xt = np.ascontiguousarray(
            xc.reshape(NC_CHUNKS, R, NH, 128).transpose(0, 3, 2, 1)
        ).astype(_BF16)
        # xn: [k, ri, ro, hcol]
        xn = np.ascontiguousarray(
            xc.reshape(NC_CHUNKS, RO, 128, H).transpose(0, 2, 1, 3)
        ).astype(_BF16)
        maps.append({"w0t": w0t, "w2s": w2s, "w1s": w1s, "xt": xt, "xn": xn})
    return maps


def _gather(results):
    out = np.empty((B, S, H), np.float32)
    xr = out.reshape(B, NC_CHUNKS, CHUNK, H)
    for core in range(CORES):
        b = core // 2
        c0 = (core % 2) * R
        oc = results[core]["out"]  # [nC, 128, RO, H]
        xr[b, :, c0:c0 + R, :] = oc.transpose(0, 2, 1, 3).reshape(NC_CHUNKS, R, H)
    sse = results[0]["sse"].astype(np.float64)
    losses = sse / LOSS_DENOM
    mean_loss = np.float32(losses.mean())
    return out, mean_loss


def kernel(x, w0, w1, w2, _trace=False):
    from concourse.bass_utils import run_bass_kernel_spmd

    if "nc" not in _nc_cache:
        _nc_cache["nc"] = _build_nc()
    nc = _nc_cache["nc"]

    in_maps = _prep_inputs(
        np.asarray(x, np.float32), np.asarray(w0, np.float32),
        np.asarray(w1, np.float32), np.asarray(w2, np.float32),
    )
    res = run_bass_kernel_spmd(nc, in_maps, list(range(CORES)), trace=_trace)
    out, mean_loss = _gather(res.results)
    if _trace:
        kernel.last_exec_time_ns = res.exec_time_ns
        kernel.last_results = res
    return out, mean_loss
